# revision 8
# baseline (speedup 1.0000x reference)
"""Multi-head attention (B=2, C=64, H=W=64, nh=8) on TRN2 NeuronCores.

Sharding: one core per batch element (2 cores); each core runs all 8 heads.
Rationale: the metric is wall-clock per call through the axon tunnel, which
is dominated by a fixed ~70 ms dispatch floor plus ~9 ms/MB of host->device
traffic.  One-core-per-batch means x is never replicated across cores, so
the per-call H2D payload is just x in bf16 (1 MB total) + tiny weights.

Host-side folds (pure data prep, no FLOPs moved off-device except the final
residual add):
  - gamma is folded into wv/bv (attention output is linear in V), so the
    device returns gamma*attn directly and the epilogue loses a multiply.
  - the residual `+ x` is applied on host in f32 (x is already on host),
    which keeps full residual precision while shipping x to the device in
    bf16 (the attention path tolerates bf16 easily: its output magnitude is
    ~1% of the residual).

Per-core pipeline (all on-chip, scores never hit HBM):
  conv1x1 (PE bf16, bias via ones-row)
  -> DRAM-bounce gathers to build Q^T/K^T [8,4096] (bf16) and V-chunked
     [128,32,33] (bf16, ones column for the softmax denominator)
     (torch .view semantics: Q[n,d] = conv[n//512, (n%512)*8 + d])
  -> S^T tiles [128m, 512n] = K Q^T on PE (bf16), exp on ACT with 1/sqrt(8)
     folded into the activation scale, bf16 output
     (no max subtraction; scores are O(1) by construction)
  -> PV via augmented V|ones bf16 matmul -> [33, 512] psum (row 32 = denom)
  -> normalize: broadcast denom via PE ones outer product, DVE reciprocal,
     pv*recip -> bf16 out.
"""

import os

os.environ.setdefault("NEURON_RT_RESET_CORES", "1")  # recover wedged cores

import numpy as np
import ml_dtypes

import concourse.bacc as bacc
import concourse.bass as bass
import concourse.tile as tile
from concourse import mybir
from concourse.bass_utils import run_bass_kernel_spmd

F32 = mybir.dt.float32
F32R = mybir.dt.float32r
BF16 = mybir.dt.bfloat16
F8E4 = mybir.dt.float8e4

B = 2
C = 64
N = 4096          # H*W
NH = 8
HD = 8            # head dim
NCORES = 2        # one core per batch element
HPC = NH          # heads per core
NBLK = N // 512   # 8 n-blocks of 512 query positions
MCHUNK = N // 128  # 32 m-chunks of 128 key positions
EXPW = 1536       # elements exp'd per ACT instruction (psum banks = EXPW/512)
SCALE = 1.0 / np.sqrt(float(HD))
NPBF16 = ml_dtypes.bfloat16
NPF8 = ml_dtypes.float8_e4m3


def _chunk_groups():
    """Partition the 32 m-chunks into groups of <= EXPW//512 for one exp each."""
    per = EXPW // 512
    groups, k = [], 0
    while k < MCHUNK:
        n = min(per, MCHUNK - k)
        groups.append(list(range(k, k + n)))
        k += n
    return groups


def _emit(tc, xb_d, wcat_d, out_d, scr):
    nc = tc.nc

    with (
        tc.tile_pool(name="persist", bufs=1) as per,
        tc.tile_pool(name="ptp", bufs=3) as ptp,
        tc.tile_pool(name="epl", bufs=3) as epl,
        tc.tile_pool(name="hdp", bufs=2) as hdp,
        tc.tile_pool(name="stp", bufs=2, space="PSUM") as stp,
        tc.tile_pool(name="accp", bufs=2, space="PSUM") as accp,
    ):
        # ---- persistent: fp32r ones row for the denominator broadcast ----
        ones8r = per.tile([1, HD], F32R)
        o8f = per.tile([1, HD], F32)
        nc.vector.memset(o8f, 1.0)
        nc.vector.tensor_copy(ones8r, o8f)  # rounds to fp32r

        # ---- conv1x1: [64,4096] = wT.T @ [65,4096] for each of q/k/v ----
        with tc.tile_pool(name="convin", bufs=1) as cin:
            x8 = cin.tile([C, N], F8E4)       # x[b] shipped as fp8e4m3
            nc.sync.dma_start(out=x8[:], in_=xb_d[:])
            xba = cin.tile([C + 1, N], BF16)  # x[b] + ones row (bias)
            nc.vector.memset(xba[C : C + 1, :], 1.0)
            nc.vector.tensor_copy(xba[0:C, :], x8[:])  # upcast fp8 -> bf16
            wc = cin.tile([C + 1, 3 * C], BF16)
            nc.sync.dma_start(out=wc[:], in_=wcat_d[:])

            with tc.tile_pool(name="convout", bufs=1) as cop:
                cqkv = [
                    cop.tile([C, N], BF16, name=f"c{t}", tag=f"c{t}")
                    for t in range(3)
                ]
                for t in range(3):
                    lhsT = wc[:, t * C : (t + 1) * C]
                    for j in range(NBLK):
                        ps = stp.tile([128, EXPW], F32, tag="st")
                        nc.tensor.matmul(
                            ps[0:C, 0:512],
                            lhsT=lhsT,
                            rhs=xba[:, j * 512 : (j + 1) * 512],
                            start=True,
                            stop=True,
                        )
                        nc.vector.tensor_copy(
                            cqkv[t][:, j * 512 : (j + 1) * 512], ps[0:C, 0:512]
                        )
                    nc.sync.dma_start(out=scr[t][:], in_=cqkv[t][:])

        # ---- per-head: re-layout gathers (torch .view semantics), attn ----
        for h in range(HPC):
            rows = slice(h * HD, (h + 1) * HD)
            qt = hdp.tile([HD, N], BF16, name=f"qt{h}", tag="qt")
            kt = hdp.tile([HD, N], BF16, name=f"kt{h}", tag="kt")
            vc = hdp.tile([128, MCHUNK, 33], BF16, name=f"vc{h}", tag="vc")

            # Q^T/K^T [d, n]: element = conv[row r, col 8t+d], n = 512r+t
            for r0 in range(0, HD, 2):
                nc.sync.dma_start(
                    out=qt[:].rearrange("d (r t) -> d r t", r=HD)[
                        :, r0 : r0 + 2, :
                    ],
                    in_=scr[0][rows, :].rearrange("r (t d) -> d r t", d=HD)[
                        :, r0 : r0 + 2, :
                    ],
                )
                nc.sync.dma_start(
                    out=kt[:].rearrange("d (r t) -> d r t", r=HD)[
                        :, r0 : r0 + 2, :
                    ],
                    in_=scr[1][rows, :].rearrange("r (t d) -> d r t", d=HD)[
                        :, r0 : r0 + 2, :
                    ],
                )
            # V chunked [i, chunk, d]: m = 128*chunk + i, chunk = 4r+tb
            nc.sync.dma_start(
                out=vc[:, :, 0:HD],
                in_=scr[2][rows, :].rearrange(
                    "r (tb i d) -> i (r tb) d", tb=4, i=128, d=HD
                ),
            )
            nc.vector.memset(vc[:, :, HD:32], 0.0)
            nc.vector.memset(vc[:, :, 32:33], 1.0)

            # ---- attention per n-block ----
            for j in range(NBLK):
                qblk = qt[:, j * 512 : (j + 1) * 512]
                acc = accp.tile([33, 512], F32, tag="acc")
                for grp in _chunk_groups():
                    st = stp.tile([128, EXPW], F32, tag="st")
                    for u, k in enumerate(grp):
                        nc.tensor.matmul(
                            st[:, u * 512 : (u + 1) * 512],
                            lhsT=kt[:, k * 128 : (k + 1) * 128],
                            rhs=qblk,
                            start=True,
                            stop=True,
                        )
                    w = len(grp) * 512
                    pt = ptp.tile([128, EXPW], BF16)
                    nc.scalar.activation(
                        pt[:, 0:w], st[:, 0:w],
                        mybir.ActivationFunctionType.Exp, scale=SCALE
                    )
                    for u, k in enumerate(grp):
                        nc.tensor.matmul(
                            acc[:, :],
                            lhsT=vc[:, k, :],
                            rhs=pt[:, u * 512 : (u + 1) * 512],
                            start=(k == 0),
                            stop=(k == MCHUNK - 1),
                        )

                # ---- epilogue: out = pv / denom (gamma pre-folded into V) ----
                sb = epl.tile([1, 512], F32R, tag="sb")
                nc.vector.tensor_copy(sb, acc[32:33, :])  # denom -> fp32r
                rb = accp.tile([33, 512], F32, tag="acc")
                nc.tensor.matmul(
                    rb[0:HD, :], lhsT=ones8r, rhs=sb, start=True, stop=True
                )
                rbs = epl.tile([HD, 512], F32, tag="rbs")
                nc.vector.reciprocal(rbs, rb[0:HD, :])
                fin = epl.tile([HD, 512], BF16, tag="fin")
                nc.vector.tensor_mul(fin, acc[0:HD, :], rbs)
                nc.sync.dma_start(
                    out=out_d[rows, j * 512 : (j + 1) * 512], in_=fin
                )


def build_bass():
    nc = bacc.Bacc("TRN2", target_bir_lowering=False, debug=False, num_devices=NCORES)
    xb_d = nc.dram_tensor("xb", [C, N], F8E4, kind="ExternalInput").ap()
    wcat_d = nc.dram_tensor("wcat", [C + 1, 3 * C], BF16, kind="ExternalInput").ap()
    out_d = nc.dram_tensor("out", [C, N], BF16, kind="ExternalOutput").ap()
    scr = [nc.dram_tensor(f"scr{t}", [C, N], BF16).ap() for t in range(3)]

    with tile.TileContext(nc) as tc:
        _emit(tc, xb_d, wcat_d, out_d, scr)
    nc.finalize()
    return nc


_NC = None


def _get_nc():
    global _NC
    if _NC is None:
        _NC = build_bass()
    return _NC


def make_in_maps(x, wq, bq, wk, bk, wv, bv, gamma):
    x = np.asarray(x, np.float32)
    g = float(np.asarray(gamma, np.float32).reshape(-1)[0])
    wcat = np.empty((C + 1, 3 * C), np.float32)
    for t, (w, bias, s) in enumerate(
        ((wq, bq, 1.0), (wk, bk, 1.0), (wv, bv, g))
    ):
        wcat[:C, t * C : (t + 1) * C] = np.asarray(w, np.float32).T * s
        wcat[C, t * C : (t + 1) * C] = np.asarray(bias, np.float32) * s
    wcat16 = wcat.astype(NPBF16)
    return [
        {
            "xb": np.ascontiguousarray(x[b].reshape(C, N)).astype(NPF8),
            "wcat": wcat16,
        }
        for b in range(NCORES)
    ]


def assemble_out(results, x):
    attn = np.stack(
        [np.asarray(results[b]["out"], dtype=np.float32) for b in range(NCORES)]
    )
    out = attn.reshape(B, C, 64, 64) + np.asarray(x, np.float32)
    return out


def kernel(x, wq, bq, wk, bk, wv, bv, gamma):
    nc = _get_nc()
    in_maps = make_in_maps(x, wq, bq, wk, bk, wv, bv, gamma)
    res = run_bass_kernel_spmd(nc, in_maps, list(range(NCORES))).results
    return assemble_out(res, x)


if __name__ == "__main__":
    rng = np.random.default_rng(0)
    x = rng.standard_normal((B, C, 64, 64), dtype=np.float32)
    wq, wk, wv = (
        rng.standard_normal((C, C), dtype=np.float32) / 8.0 for _ in range(3)
    )
    bq, bk, bv = (
        rng.standard_normal((C,), dtype=np.float32) * 0.01 for _ in range(3)
    )
    gamma = rng.random((1,), dtype=np.float32)
    out = kernel(x, wq, bq, wk, bk, wv, bv, gamma)
    print(out.shape, out.dtype)


# revision 10
# speedup vs baseline: 1.0289x; 1.0289x over previous
"""Multi-head attention (B=2, C=64, H=W=64, nh=8) on TRN2 NeuronCores.

Sharding: one core per batch element (2 cores); each core runs all 8 heads.
Rationale: the metric is wall-clock per call through the axon tunnel, which
is dominated by a fixed ~32-35 ms dispatch floor plus ~9 ms/MB of
host->device traffic (measured; on-device exec is only ~4 ms of a ~40 ms
call).  One-core-per-batch means x is never replicated across cores, so the
per-call H2D payload is just x in fp8 (0.5 MB total) + tiny bf16 weights.
The old 8-core layout shipped 12.7 MB/call (x replicated 4x in f32 + f32
scratch) and measured 183-253 ms.

Host-side folds (pure data prep, no FLOPs moved off-device except the final
residual add):
  - gamma is folded into wv/bv (attention output is linear in V), so the
    device returns gamma*attn directly and the epilogue loses a multiply.
  - the residual `+ x` is applied on host in f32 (x is already on host),
    which keeps full residual precision while shipping x to the device in
    fp8e4m3 (upcast to bf16 on-chip for the conv).  The attention path
    tolerates fp8 x easily -- its output magnitude is ~1% of the residual;
    measured end-to-end relerr 8.8e-3 vs the 2e-2 gate (bf16 x: 9.5e-4).

Per-core pipeline (all on-chip, scores never hit HBM):
  fp8 x -> bf16 upcast (DVE), conv1x1 (PE bf16, bias via ones-row)
  -> DRAM-bounce gathers to build Q^T/K^T [8,4096] (bf16) and V-chunked
     [128,32,33] (bf16, ones column for the softmax denominator)
     (torch .view semantics: Q[n,d] = conv[n//512, (n%512)*8 + d])
  -> S^T tiles [128m, 512n] = K Q^T on PE (bf16), exp on ACT with 1/sqrt(8)
     folded into the activation scale, bf16 output
     (no max subtraction; scores are O(1) by construction)
  -> PV via augmented V|ones bf16 matmul -> [33, 512] psum (row 32 = denom)
  -> normalize: broadcast denom via PE ones outer product, DVE reciprocal,
     pv*recip -> bf16 out.
"""

import os

os.environ.setdefault("NEURON_RT_RESET_CORES", "1")  # recover wedged cores

import numpy as np
import ml_dtypes

import concourse.bacc as bacc
import concourse.bass as bass
import concourse.tile as tile
from concourse import mybir
from concourse.bass_utils import run_bass_kernel_spmd

F32 = mybir.dt.float32
F32R = mybir.dt.float32r
BF16 = mybir.dt.bfloat16
F8E4 = mybir.dt.float8e4

B = 2
C = 64
N = 4096          # H*W
NH = 8
HD = 8            # head dim
NCORES = 2        # one core per batch element
HPC = NH          # heads per core
NBLK = N // 512   # 8 n-blocks of 512 query positions
MCHUNK = N // 128  # 32 m-chunks of 128 key positions
EXPW = 1536       # elements exp'd per ACT instruction (psum banks = EXPW/512)
SCALE = 1.0 / np.sqrt(float(HD))
NPBF16 = ml_dtypes.bfloat16
NPF8 = ml_dtypes.float8_e4m3


def _chunk_groups():
    """Partition the 32 m-chunks into groups of <= EXPW//512 for one exp each."""
    per = EXPW // 512
    groups, k = [], 0
    while k < MCHUNK:
        n = min(per, MCHUNK - k)
        groups.append(list(range(k, k + n)))
        k += n
    return groups


def _emit(tc, xb_d, wcat_d, out_d, scr):
    nc = tc.nc

    with (
        tc.tile_pool(name="persist", bufs=1) as per,
        tc.tile_pool(name="ptp", bufs=3) as ptp,
        tc.tile_pool(name="epl", bufs=3) as epl,
        tc.tile_pool(name="hdp", bufs=2) as hdp,
        tc.tile_pool(name="stp", bufs=2, space="PSUM") as stp,
        tc.tile_pool(name="accp", bufs=2, space="PSUM") as accp,
    ):
        # ---- persistent: fp32r ones row for the denominator broadcast ----
        ones8r = per.tile([1, HD], F32R)
        o8f = per.tile([1, HD], F32)
        nc.vector.memset(o8f, 1.0)
        nc.vector.tensor_copy(ones8r, o8f)  # rounds to fp32r

        # ---- conv1x1: [64,4096] = wT.T @ [65,4096] for each of q/k/v ----
        with tc.tile_pool(name="convin", bufs=1) as cin:
            x8 = cin.tile([C, N], F8E4)       # x[b] shipped as fp8e4m3
            nc.sync.dma_start(out=x8[:], in_=xb_d[:])
            xba = cin.tile([C + 1, N], BF16)  # x[b] + ones row (bias)
            nc.vector.memset(xba[C : C + 1, :], 1.0)
            nc.vector.tensor_copy(xba[0:C, :], x8[:])  # upcast fp8 -> bf16
            wc = cin.tile([C + 1, 3 * C], BF16)
            nc.sync.dma_start(out=wc[:], in_=wcat_d[:])

            with tc.tile_pool(name="convout", bufs=1) as cop:
                cqkv = [
                    cop.tile([C, N], BF16, name=f"c{t}", tag=f"c{t}")
                    for t in range(3)
                ]
                for t in range(3):
                    lhsT = wc[:, t * C : (t + 1) * C]
                    for j in range(NBLK):
                        ps = stp.tile([128, EXPW], F32, tag="st")
                        nc.tensor.matmul(
                            ps[0:C, 0:512],
                            lhsT=lhsT,
                            rhs=xba[:, j * 512 : (j + 1) * 512],
                            start=True,
                            stop=True,
                        )
                        nc.vector.tensor_copy(
                            cqkv[t][:, j * 512 : (j + 1) * 512], ps[0:C, 0:512]
                        )
                    nc.sync.dma_start(out=scr[t][:], in_=cqkv[t][:])

        # ---- per-head: re-layout gathers (torch .view semantics), attn ----
        for h in range(HPC):
            rows = slice(h * HD, (h + 1) * HD)
            qt = hdp.tile([HD, N], BF16, name=f"qt{h}", tag="qt")
            kt = hdp.tile([HD, N], BF16, name=f"kt{h}", tag="kt")
            vc = hdp.tile([128, MCHUNK, 33], BF16, name=f"vc{h}", tag="vc")

            # Q^T/K^T [d, n]: element = conv[row r, col 8t+d], n = 512r+t
            for r0 in range(0, HD, 2):
                nc.sync.dma_start(
                    out=qt[:].rearrange("d (r t) -> d r t", r=HD)[
                        :, r0 : r0 + 2, :
                    ],
                    in_=scr[0][rows, :].rearrange("r (t d) -> d r t", d=HD)[
                        :, r0 : r0 + 2, :
                    ],
                )
                nc.sync.dma_start(
                    out=kt[:].rearrange("d (r t) -> d r t", r=HD)[
                        :, r0 : r0 + 2, :
                    ],
                    in_=scr[1][rows, :].rearrange("r (t d) -> d r t", d=HD)[
                        :, r0 : r0 + 2, :
                    ],
                )
            # V chunked [i, chunk, d]: m = 128*chunk + i, chunk = 4r+tb
            nc.sync.dma_start(
                out=vc[:, :, 0:HD],
                in_=scr[2][rows, :].rearrange(
                    "r (tb i d) -> i (r tb) d", tb=4, i=128, d=HD
                ),
            )
            nc.vector.memset(vc[:, :, HD:32], 0.0)
            nc.vector.memset(vc[:, :, 32:33], 1.0)

            # ---- attention per n-block ----
            for j in range(NBLK):
                qblk = qt[:, j * 512 : (j + 1) * 512]
                acc = accp.tile([33, 512], F32, tag="acc")
                for grp in _chunk_groups():
                    st = stp.tile([128, EXPW], F32, tag="st")
                    for u, k in enumerate(grp):
                        nc.tensor.matmul(
                            st[:, u * 512 : (u + 1) * 512],
                            lhsT=kt[:, k * 128 : (k + 1) * 128],
                            rhs=qblk,
                            start=True,
                            stop=True,
                        )
                    w = len(grp) * 512
                    pt = ptp.tile([128, EXPW], BF16)
                    nc.scalar.activation(
                        pt[:, 0:w], st[:, 0:w],
                        mybir.ActivationFunctionType.Exp, scale=SCALE
                    )
                    for u, k in enumerate(grp):
                        nc.tensor.matmul(
                            acc[:, :],
                            lhsT=vc[:, k, :],
                            rhs=pt[:, u * 512 : (u + 1) * 512],
                            start=(k == 0),
                            stop=(k == MCHUNK - 1),
                        )

                # ---- epilogue: out = pv / denom (gamma pre-folded into V) ----
                sb = epl.tile([1, 512], F32R, tag="sb")
                nc.vector.tensor_copy(sb, acc[32:33, :])  # denom -> fp32r
                rb = accp.tile([33, 512], F32, tag="acc")
                nc.tensor.matmul(
                    rb[0:HD, :], lhsT=ones8r, rhs=sb, start=True, stop=True
                )
                rbs = epl.tile([HD, 512], F32, tag="rbs")
                nc.vector.reciprocal(rbs, rb[0:HD, :])
                fin = epl.tile([HD, 512], BF16, tag="fin")
                nc.vector.tensor_mul(fin, acc[0:HD, :], rbs)
                nc.sync.dma_start(
                    out=out_d[rows, j * 512 : (j + 1) * 512], in_=fin
                )


def build_bass():
    nc = bacc.Bacc("TRN2", target_bir_lowering=False, debug=False, num_devices=NCORES)
    xb_d = nc.dram_tensor("xb", [C, N], F8E4, kind="ExternalInput").ap()
    wcat_d = nc.dram_tensor("wcat", [C + 1, 3 * C], BF16, kind="ExternalInput").ap()
    out_d = nc.dram_tensor("out", [C, N], BF16, kind="ExternalOutput").ap()
    scr = [nc.dram_tensor(f"scr{t}", [C, N], BF16).ap() for t in range(3)]

    with tile.TileContext(nc) as tc:
        _emit(tc, xb_d, wcat_d, out_d, scr)
    nc.finalize()
    return nc


_NC = None


def _get_nc():
    global _NC
    if _NC is None:
        _NC = build_bass()
    return _NC


def make_in_maps(x, wq, bq, wk, bk, wv, bv, gamma):
    x = np.asarray(x, np.float32)
    g = float(np.asarray(gamma, np.float32).reshape(-1)[0])
    wcat = np.empty((C + 1, 3 * C), np.float32)
    for t, (w, bias, s) in enumerate(
        ((wq, bq, 1.0), (wk, bk, 1.0), (wv, bv, g))
    ):
        wcat[:C, t * C : (t + 1) * C] = np.asarray(w, np.float32).T * s
        wcat[C, t * C : (t + 1) * C] = np.asarray(bias, np.float32) * s
    wcat16 = wcat.astype(NPBF16)
    return [
        {
            "xb": np.ascontiguousarray(x[b].reshape(C, N)).astype(NPF8),
            "wcat": wcat16,
        }
        for b in range(NCORES)
    ]


def assemble_out(results, x):
    attn = np.stack(
        [np.asarray(results[b]["out"], dtype=np.float32) for b in range(NCORES)]
    )
    out = attn.reshape(B, C, 64, 64) + np.asarray(x, np.float32)
    return out


def kernel(x, wq, bq, wk, bk, wv, bv, gamma):
    nc = _get_nc()
    in_maps = make_in_maps(x, wq, bq, wk, bk, wv, bv, gamma)
    res = run_bass_kernel_spmd(nc, in_maps, list(range(NCORES))).results
    return assemble_out(res, x)


if __name__ == "__main__":
    rng = np.random.default_rng(0)
    x = rng.standard_normal((B, C, 64, 64), dtype=np.float32)
    wq, wk, wv = (
        rng.standard_normal((C, C), dtype=np.float32) / 8.0 for _ in range(3)
    )
    bq, bk, bv = (
        rng.standard_normal((C,), dtype=np.float32) * 0.01 for _ in range(3)
    )
    gamma = rng.random((1,), dtype=np.float32)
    out = kernel(x, wq, bq, wk, bk, wv, bv, gamma)
    print(out.shape, out.dtype)
